# revision 9
# baseline (speedup 1.0000x reference)
"""Trainium2 Bass kernel for nn_BiLingual (dual embedding gather + cAddTanh pool).

Computes, for two embedding tables:
    out[t, b, :] = sum_{j=0}^{S-2} tanh(W_t[idx_t[b, j]] + W_t[idx_t[b, j+1]])

Sharding: data-parallel over batch. Each of the 8 cores handles 8 batch rows
for BOTH tables; tables are replicated (host-cast to bf16).

Bottleneck analysis (HW-measured): the SWDGE dma_gather ucode costs ~9.3 ns
per gathered index on the Pool engine (descriptor generation on one Q7 core
pair), which is the hard floor (~305 us for 32768 positions/core). This
kernel gathers each position exactly once (contiguous 2048-position rows, no
overlap groups) and hides all compute under the gather stream:

  1. dma_gather (gpsimd): 8 calls per core (2 sequence rows each, 4112
     indices: 4096 real + 16 guard).  Positions land contiguously:
     position j of a row -> (partition j%128, group j//128), 16 groups/row.
     Each call writes a private 33-group region (32 real + 1 junk group that
     absorbs the guard slots and AP rounding), so calls never overlap and
     stream back-to-back on the Pool engine.
     int16 index range handled by biasing: base = W[32768:] and signed
     idx' = idx - 32768 in [-32768, 17231]; 16 zero guards at the end of
     each call keep the ucode's trailing-negative trim from eating real
     indices (guard rows land in the junk group).
  2. PE pair-sum, two accumulating matmuls per chunk (bf16):
       shift: A[p] = E[p] + E[p+1]        (within-group pairs, p < 127)
       wrap:  A[127] += E_next_group[0]   (cross-group pair at p = 127)
     The wrap matmul is skipped for group 15 (no next group within the row);
     that slot is masked out of the reduce instead.
  3. ACT tanh PSUM -> SBUF bf16.
  4. PE masked ones-matmul reduces tanh values over valid slots into a
     [16, 256] PSUM accumulator (partition = table*8 + local_row).
     Masks: groups 0..14 all 128 pairs valid; group 15 has 127 (pair 2047
     does not exist).
"""
import os

import numpy as np
import ml_dtypes

from concourse import bacc, mybir
import concourse.tile as tile
from concourse.bass_utils import run_bass_kernel_spmd

P = 128
B, S, V, D = 64, 2048, 50000, 256
N_CORES = 8
B_LOC = B // N_CORES           # 8 batch rows per core
NG = S // P                    # 16 groups of 128 positions per row
NROW = 2 * B_LOC               # 16 (table, local row) pairs per core
SPLIT = 32768
GUARD = 16                     # guard indices per call (trailing-trim safety)
# rows per gather call; the last two calls are single-row to shrink the
# compute tail that runs after the final gather completes
CALL_PLAN = [2, 2, 2, 2, 2, 2, 2, 1, 1]
N_CALLS = len(CALL_PLAN)
CALL_ROW0 = [sum(CALL_PLAN[:i]) for i in range(N_CALLS)]  # first row16 of call
ROW_CALL = [i for i, n in enumerate(CALL_PLAN) for _ in range(n)]
CALL_IDX = [n * S + GUARD for n in CALL_PLAN]   # indices per call
ICOLS = [n // 16 for n in CALL_IDX]             # idx columns per call
ICOL0 = [sum(ICOLS[:i]) for i in range(N_CALLS)]
ICOLS_TOT = sum(ICOLS)
CALL_G = [n * NG + 1 for n in CALL_PLAN]        # groups per call region (+1 junk)
CALL_G0 = [sum(CALL_G[:i]) for i in range(N_CALLS)]
E_G = sum(CALL_G)              # groups in the E buffer
# chunks per row: (first group, n groups, has wrap matmul)
CHUNKS = [(0, 4, True), (4, 4, True), (8, 4, True), (12, 3, True), (15, 1, False)]

_last_results = None           # set by _run for test harness introspection


def _build_shiftT():
    # lhsT for A = M2 @ E with M2[m,m]=1, M2[m,m+1]=1  =>  lhsT[k,m] = M2[m,k]
    m = np.zeros((P, P), dtype=np.float32)
    k = np.arange(P)
    m[k, k] = 1.0
    m[k[1:], k[1:] - 1] = 1.0
    return m.astype(ml_dtypes.bfloat16)


def _build_wrapT():
    # lhsT with [k=0, m=127] = 1: adds rhs row 0 (next group's position 0)
    # into output partition 127, completing pair (128g+127, 128g+128).
    m = np.zeros((P, P), dtype=np.float32)
    m[0, P - 1] = 1.0
    return m.astype(ml_dtypes.bfloat16)


def _build_red_masks():
    # red[:, (row16*2 + ty)*16 : +16]: column row16 holds mask_ty, rest 0.
    # ty=0 (groups 0..14): all 128 pairs valid; ty=1 (group 15): p < 127.
    red = np.zeros((P, NROW * 2 * 16), dtype=np.float32)
    masks = [
        np.ones(P, dtype=np.float32),
        (np.arange(P) < P - 1).astype(np.float32),
    ]
    for row16 in range(NROW):
        for ty in range(2):
            red[:, (row16 * 2 + ty) * 16 + row16] = masks[ty]
    return red.astype(ml_dtypes.bfloat16)


def _split_multi_waits(nc, max_waits=1):
    """Walrus rejects instructions carrying too many sync waits; hoist excess
    waits onto same-engine NOPs inserted just before the instruction (engine
    program order makes this equivalent)."""
    for bb in nc.main_func.blocks:
        idx = 0
        while idx < len(bb.instructions):
            ins = bb.instructions[idx]
            si = ins.sync_info
            if si is not None and si.on_wait and len(si.on_wait) > max_waits:
                waits = list(si.on_wait)
                extra, keep = waits[:-max_waits], waits[-max_waits:]
                for w0 in range(0, len(extra), max_waits):
                    nop = mybir.InstNoOp(
                        name=nc.get_next_instruction_name(), ins=[], outs=[]
                    )
                    nop.engine = ins.engine
                    nop.sync_info = mybir.SyncInfo(
                        on_wait=extra[w0 : w0 + max_waits], on_update=[]
                    )
                    nc.register_instruction(nop)
                    bb.instructions.insert(idx, nop)
                    idx += 1
                si.on_wait = keep
            idx += 1


def _build_program():
    # 2-row gather calls need ~265 descriptors per SDMA ring; the default
    # 16 KiB/partition carveout holds ~256, so double it.
    nc = bacc.Bacc(None, target_bir_lowering=False, dynamic_dma_scratch_size=32768)
    bf16 = mybir.dt.bfloat16
    Wp = nc.declare_dram_parameter("W_pri", [V, D], bf16, isOutput=False)
    Ws = nc.declare_dram_parameter("W_sec", [V, D], bf16, isOutput=False)
    idxA = nc.declare_dram_parameter(
        "idxA", [P, ICOLS_TOT], mybir.dt.int16, isOutput=False
    )
    shiftT = nc.declare_dram_parameter("shiftT", [P, P], bf16, isOutput=False)
    wrapT = nc.declare_dram_parameter("wrapT", [P, P], bf16, isOutput=False)
    red = nc.declare_dram_parameter("red", [P, NROW * 2 * 16], bf16, isOutput=False)
    out = nc.declare_dram_parameter("out", [NROW, D], mybir.dt.float32, isOutput=True)

    with tile.TileContext(nc) as tc:
        with (
            tc.tile_pool(name="const", bufs=1) as const,
            tc.tile_pool(name="ebuf", bufs=1) as ebuf,
            tc.tile_pool(name="tbuf", bufs=3) as tbuf,
            tc.tile_pool(name="psA", bufs=3, space="PSUM") as psA,
            tc.tile_pool(name="psR", bufs=1, space="PSUM") as psR,
            tc.tile_pool(name="osb", bufs=1) as osb,
        ):
            # warm-up: a 128-index dummy gather pays the gather ucode's ~6 us
            # first-use IRAM load while the real idx table is still uploading.
            # Its indices come from a memset-zeroed tile (no DMA dependency).
            iZ = const.tile([P, 8], mybir.dt.int16)
            nc.gpsimd.memset(iZ[:], 0)
            eZ = const.tile([P, 1, D], bf16)
            nc.gpsimd.dma_gather(
                out_ap=eZ[:],
                in_ap=Wp[SPLIT:, :],
                idxs_ap=iZ[:],
                num_idxs=P,
                num_idxs_reg=P,
                elem_size=D,
            )

            iA = const.tile([P, ICOLS_TOT], mybir.dt.int16)
            nc.sync.dma_start(out=iA[:], in_=idxA[:])
            shift_t = const.tile([P, P], bf16)
            nc.sync.dma_start(out=shift_t[:], in_=shiftT[:])
            wrap_t = const.tile([P, P], bf16)
            nc.sync.dma_start(out=wrap_t[:], in_=wrapT[:])
            red_t = const.tile([P, NROW * 2 * 16], bf16)
            nc.sync.dma_start(out=red_t[:], in_=red[:])

            ebig = ebuf.tile([P, E_G, D], bf16)
            ef = ebig[:].rearrange("p g d -> p (g d)")

            for c in range(N_CALLS):
                W = Wp if CALL_ROW0[c] < B_LOC else Ws
                nc.gpsimd.dma_gather(
                    out_ap=ebig[:, CALL_G0[c] : CALL_G0[c] + CALL_G[c], :],
                    in_ap=W[SPLIT:, :],
                    idxs_ap=iA[:, ICOL0[c] : ICOL0[c] + ICOLS[c]],
                    num_idxs=CALL_IDX[c],
                    num_idxs_reg=CALL_IDX[c],
                    elem_size=D,
                    # >64 descs/ring exceeds the single-packet ceiling; let
                    # each descriptor form its own packet.
                    single_packet=False,
                )

            acc = psR.tile([NROW, D], mybir.dt.float32, space="PSUM")
            n_red = NROW * NG
            red_i = 0

            for t in range(2):
                for r in range(B_LOC):
                    row16 = t * B_LOC + r
                    call = ROW_CALL[row16]
                    gbase = CALL_G0[call] + (row16 - CALL_ROW0[call]) * NG
                    for c0, ng, wrap in CHUNKS:
                        ncol = ng * D
                        a = psA.tile([P, 4 * D], mybir.dt.float32, space="PSUM")
                        for h0 in range(0, ng, 2):
                            nh = min(2, ng - h0)
                            nc.tensor.matmul(
                                out=a[:, h0 * D : (h0 + nh) * D],
                                lhsT=shift_t[:],
                                rhs=ef[
                                    :,
                                    (gbase + c0 + h0) * D : (gbase + c0 + h0 + nh) * D,
                                ],
                                start=True,
                                stop=not wrap,
                            )
                        if wrap:
                            for h0 in range(0, ng, 2):
                                nh = min(2, ng - h0)
                                nc.tensor.matmul(
                                    out=a[:, h0 * D : (h0 + nh) * D],
                                    lhsT=wrap_t[:],
                                    rhs=ef[
                                        :,
                                        (gbase + c0 + h0 + 1) * D : (
                                            gbase + c0 + h0 + 1 + nh
                                        )
                                        * D,
                                    ],
                                    start=False,
                                    stop=True,
                                )
                        tt = tbuf.tile([P, 4 * D], bf16)
                        nc.scalar.activation(
                            tt[:, :ncol],
                            a[:, :ncol],
                            mybir.ActivationFunctionType.Tanh,
                        )
                        for gi in range(ng):
                            ty = 1 if c0 + gi == NG - 1 else 0
                            nc.tensor.matmul(
                                out=acc[:],
                                lhsT=red_t[
                                    :, (row16 * 2 + ty) * 16 : (row16 * 2 + ty + 1) * 16
                                ],
                                rhs=tt[:, gi * D : (gi + 1) * D],
                                start=(red_i == 0),
                                stop=(red_i == n_red - 1),
                            )
                            red_i += 1

            res_sb = osb.tile([NROW, D], mybir.dt.float32)
            nc.scalar.copy(out=res_sb[:], in_=acc[:])
            nc.sync.dma_start(out=out[:], in_=res_sb[:])

    nc.compile()
    _split_multi_waits(nc)
    return nc


def _host_prep(inputs_pri, inputs_sec, W_pri, W_sec):
    ip = np.asarray(inputs_pri).astype(np.int64, copy=False)
    is_ = np.asarray(inputs_sec).astype(np.int64, copy=False)
    wp = np.asarray(W_pri, dtype=np.float32).astype(ml_dtypes.bfloat16)
    ws = np.asarray(W_sec, dtype=np.float32).astype(ml_dtypes.bfloat16)
    wp = np.ascontiguousarray(wp)
    ws = np.ascontiguousarray(ws)
    shiftT = _build_shiftT()
    wrapT = _build_wrapT()
    red = _build_red_masks()

    in_maps = []
    for k in range(N_CORES):
        idxA = np.zeros((P, ICOLS_TOT), dtype=np.int16)
        for c in range(N_CALLS):
            rows = []
            for j in range(CALL_PLAN[c]):
                row16 = CALL_ROW0[c] + j
                idx = ip if row16 < B_LOC else is_
                rows.append(idx[k * B_LOC + row16 % B_LOC])
            stream = np.concatenate(rows + [np.full(GUARD, SPLIT, np.int64)])
            stream = (stream - SPLIT).astype(np.int16)  # guards -> 0
            wrapped = np.tile(stream.reshape(-1, 16).T, (8, 1))
            idxA[:, ICOL0[c] : ICOL0[c] + ICOLS[c]] = wrapped
        in_maps.append(
            {
                "W_pri": wp,
                "W_sec": ws,
                "idxA": idxA,
                "shiftT": shiftT,
                "wrapT": wrapT,
                "red": red,
            }
        )
    return in_maps


def _run(inputs_pri, inputs_sec, W_pri, W_sec, trace=False):
    global _last_results
    nc = _build_program()
    in_maps = _host_prep(inputs_pri, inputs_sec, W_pri, W_sec)
    res = run_bass_kernel_spmd(nc, in_maps, list(range(N_CORES)), trace=trace)
    _last_results = res
    out = np.empty((2, B, D), dtype=np.float32)
    for k in range(N_CORES):
        o = res.results[k]["out"]  # [16, 256]
        out[0, k * B_LOC : (k + 1) * B_LOC] = o[:B_LOC]
        out[1, k * B_LOC : (k + 1) * B_LOC] = o[B_LOC:]
    return out


def kernel(inputs_pri, inputs_sec, W_pri, W_sec):
    trace = bool(int(os.environ.get("KERNEL_TRACE", "0")))
    return _run(inputs_pri, inputs_sec, W_pri, W_sec, trace=trace)
